# revision 37
# baseline (speedup 1.0000x reference)
"""Trainium2 Bass kernel for nn_DecoderAttention (Bahdanau attention + LSTM decoder).

Data-parallel over batch: B=128 split across 8 NeuronCores (16 batches/core).
All FLOPs run on device; the host only reshuffles layouts (transpose / dtype
cast / weight concat with bias rows folded in as an extra contraction row).

v2 design (cost-model-guided):
  - kproj on PE in fp8e4 with MatmulPerfMode.DoubleRow: K=200 packed as
    [100 partitions x 2] pairs -> one pass at 0.5 cycles/col (4x fewer PE
    column-streams than the bf16 K-split version).
  - e = tanh(kproj + qproj) on ACT, written as fp8e4 pairs [100, 2, T] so the
    scores matvec is also a DoubleRow fp8 matmul (0.5 cyc/col, single pass).
  - scores -> exp (ACT, accum_out Z partials per 512-chunk), Z tree-add +
    reciprocal + p-normalize on DVE (so context needs no epilogue scaling).
  - context computed FLIPPED: encN chunk is the stationary operand, pT column
    the moving one -> N=1 matmuls (engine cost ~0) instead of N=200 streams.
    All 32 accumulation chains share one PSUM bank: a single start=True zeroes
    the 2KB zero-region once, everything else accumulates (start=False).
  - per-wave software pipeline: wave w's transposes+context are emitted one
    wave late so PE never stalls ahead of the encN DMAs or ACT exp.
  - decoder: gates host-permuted to [i,f,o,g] so the three sigmoids take 2 ACT
    instructions; gates2 = (wxr*x)+G0 fused in one DVE scalar_tensor_tensor;
    MLP in bf16; final y = o2t^T @ w3 gives [NB,1] directly (no y transpose,
    feedback column doubles as the output buffer).
"""

import numpy as np
import ml_dtypes

B, T, H = 128, 2048, 200
NCORES = 8
NB = B // NCORES  # 16 batches per core
NSTEPS = 5
G4 = 4 * H  # 800 gate width
KP = 128  # DoubleRow pair offset: contraction padded 200 -> 256 = 2*128
H0, H1 = 128, H - 128  # 128 + 72 chunks everywhere
NCH = T // 128  # 16 context t-chunks
NW = NB // 4  # 4 waves of 4 batches

_CACHE = {}

BF16 = ml_dtypes.bfloat16
F8 = ml_dtypes.float8_e4m3


def _build_module():
    """Build the Bass module (same NEFF for all 8 cores)."""
    from contextlib import ExitStack

    import concourse.bass as bass
    import concourse.tile as tile
    from concourse import bacc, mybir
    from concourse.masks import make_identity

    dt = mybir.dt
    AF = mybir.ActivationFunctionType
    OP = mybir.AluOpType
    AX = mybir.AxisListType
    PM = mybir.MatmulPerfMode

    nc = bacc.Bacc(
        "TRN2",
        target_bir_lowering=False,
        debug=False,
        num_devices=NCORES,
    )

    # ---- DRAM tensors (per-core shards; weights replicated) ----
    d_encTD = nc.dram_tensor("encTD", [NB, 128, 2, T], dt.float8e4, kind="ExternalInput").ap()
    d_encN = nc.dram_tensor("encN", [NB, 128, NCH, H], dt.float8e4, kind="ExternalInput").ap()
    d_qT = nc.dram_tensor("qT", [H, NB], dt.bfloat16, kind="ExternalInput").ap()
    d_c0 = nc.dram_tensor("c0s", [NB, H], dt.float32, kind="ExternalInput").ap()
    d_x0 = nc.dram_tensor("x0s", [NB, 1], dt.float32, kind="ExternalInput").ap()
    d_UaTD = nc.dram_tensor("UaTD", [128, 2, 208], dt.float8e4, kind="ExternalInput").ap()
    d_WaT = nc.dram_tensor("WaT", [H, H], dt.bfloat16, kind="ExternalInput").ap()
    d_qb = nc.dram_tensor("qb", [H, 1], dt.float32, kind="ExternalInput").ap()
    d_VaD = nc.dram_tensor("VaD", [128, 2, 16], dt.float8e4, kind="ExternalInput").ap()
    d_WihcT = nc.dram_tensor("WihcT", [H + 1, G4], dt.bfloat16, kind="ExternalInput").ap()
    d_WhhT = nc.dram_tensor("WhhT", [H, G4], dt.bfloat16, kind="ExternalInput").ap()
    d_wxr = nc.dram_tensor("wxr", [NB, G4], dt.bfloat16, kind="ExternalInput").ap()
    d_W1T = nc.dram_tensor("W1T", [H + 1, 100], dt.bfloat16, kind="ExternalInput").ap()
    d_W2T = nc.dram_tensor("W2T", [101, 50], dt.bfloat16, kind="ExternalInput").ap()
    d_W3T = nc.dram_tensor("W3T", [51, 1], dt.bfloat16, kind="ExternalInput").ap()
    # ones rows for the bias-row (aug) trick; DMA'd because compute engines
    # cannot write at non-32-aligned partition offsets
    d_ones_b = nc.dram_tensor("ones_b", [1, NB], dt.bfloat16, kind="ExternalInput").ap()
    d_y = nc.dram_tensor("y", [NB, NSTEPS], dt.float32, kind="ExternalOutput").ap()

    with tile.TileContext(nc) as tc, ExitStack() as ctx:
        # ---------- persistent pools ----------
        wpool = ctx.enter_context(tc.tile_pool(name="weights", bufs=1))
        spool = ctx.enter_context(tc.tile_pool(name="smalls", bufs=1))

        # identity for PE transposes (bf16 everywhere)
        id_bf = wpool.tile([128, 128], dt.bfloat16)
        make_identity(nc, id_bf[:])

        # warmup: a tiny Tanh pins the exp_and_others table set (exp+tanh+
        # relu) without a second load before the first real tanh
        wt_a = spool.tile([1, 8], dt.float32)
        nc.vector.memset(wt_a[:], 0.0)
        wt_b = spool.tile([1, 8], dt.float32)
        nc.scalar.activation(wt_b[:], wt_a[:], AF.Tanh)

        # attention weights on SP right behind encTD(0) (emitted at the top
        # of the batch loop): the ACT queue stays free for the table load
        wa0 = wpool.tile([H0, H], dt.bfloat16)
        wa1 = wpool.tile([H1, H], dt.bfloat16)
        ua_d = wpool.tile([128, 2, 208], dt.float8e4)
        qt0 = wpool.tile([H0, NB], dt.bfloat16)
        qt1 = wpool.tile([H1, NB], dt.bfloat16)
        qb0 = wpool.tile([H0, 1], dt.float32)
        qb1 = wpool.tile([H1, 1], dt.float32)
        va_d = wpool.tile([128, 2, 16], dt.float8e4)

        # decoder weights (allocated now, DMA'd after the first few encTD
        # loads so attention keeps HBM priority)
        wihc0 = wpool.tile([H0, G4], dt.bfloat16)
        wihc1 = wpool.tile([H1 + 1, G4], dt.bfloat16)
        whh0 = wpool.tile([H0, G4], dt.bfloat16)
        whh1 = wpool.tile([H1, G4], dt.bfloat16)
        wxr_sb = wpool.tile([NB, G4], dt.bfloat16)
        w1t0 = wpool.tile([H0, 100], dt.bfloat16)
        w1t1 = wpool.tile([H1 + 1, 100], dt.bfloat16)
        w2t = wpool.tile([101, 50], dt.bfloat16)
        w3t = wpool.tile([51, 1], dt.bfloat16)
        c0_sb = spool.tile([NB, H], dt.float32)
        x_sb = spool.tile([NB, 1], dt.float32)
        y_sb = spool.tile([NB, NSTEPS], dt.float32)

        # qproj tiles (computed inside the loop at b==0, after the weight
        # DMAs are issued, reusing the scores PSUM pool)
        qproj = [
            spool.tile([msz, NB], dt.float32, name=f"qp{m}")
            for m, msz in ((0, H0), (1, H1))
        ]

        # ---------- attention pipeline ----------
        import bass_rust as _br

        etd_pool = ctx.enter_context(tc.tile_pool(name="etd_pool", bufs=3))
        encN_pool = ctx.enter_context(tc.tile_pool(name="encN_pool", bufs=10))
        p_pool = ctx.enter_context(tc.tile_pool(name="p_pool", bufs=3))
        pn_pool = ctx.enter_context(tc.tile_pool(name="pn_pool", bufs=4))
        dve_sm = ctx.enter_context(tc.tile_pool(name="dve_sm", bufs=2))

        ct0 = spool.tile([H0, NB], dt.bfloat16)
        ct1 = spool.tile([H1 + 1, NB], dt.bfloat16)  # row 72 = ones (bias row)
        nc.sync.dma_start(ct1[H1 : H1 + 1, :], d_ones_b[:, :])

        # all-ones matrix: Z broadcast via ones^T @ p (column sums on every
        # partition), so 1/Z needs no partition broadcast
        ones_bf = wpool.tile([128, 128], dt.bfloat16)
        nc.vector.memset(ones_bf[:], 1.0)

        # e pair tiles, round-robin; pair-1 rows 72:128 are permanently zero
        # (contraction padding for the DoubleRow scores matvec)
        NEC = 5
        ec_bufs = [
            spool.tile([128, 2, T], dt.float8e4, name=f"ec{i}") for i in range(NEC)
        ]
        for i in range(NEC):
            # partition offset must be 32-aligned; rows 64:72 are overwritten
            # by the first tanh before any scores matmul reads them. On Pool:
            # the DVE queue must stay clear for the first qproj bias add
            nc.gpsimd.memset(ec_bufs[i][64:128, 1, :], 0.0)
        en_tiles = []
        pn_tiles = {}
        ctx_started = [False]

        g0_ps = ctx.enter_context(tc.tile_pool(name="g0_psum", bufs=1, space="PSUM"))
        gp = g0_ps.tile([NB, 1024], dt.float32, tag="g0")
        with (
            tc.tile_pool(name="kp_psum", bufs=2, space="PSUM") as kp_ps,
            tc.tile_pool(name="sc_psum", bufs=1, space="PSUM") as sc_ps,
            tc.tile_pool(name="ctx_psum", bufs=1, space="PSUM") as ctx_ps,
        ):
            # one bank holds the 32 context accumulation chains (cols 0:32)
            # AND the 16 per-batch Z tiles (cols 32+16b): Z(0) start=True
            # zeroes the bank once, everything else accumulates lazily
            ctp = ctx_ps.tile([128, 512], dt.float32, tag="ctx")

            def emit_scores(b):
                """transposed scores for batch b: e chunk stationary, Va
                moving -> scT[t, 1] per chunk lands as a column of scb
                (t-major, matching the encN chunk layout). Z shares scb's
                PSUM bank (cols 16:32), written strictly after exp reads."""
                ecb = ec_bufs[b % NEC]
                scb = sc_ps.tile([128, 512], dt.float32, tag="sc")
                for c in range(NCH):
                    nc.tensor.matmul(
                        scb[:, c : c + 1],
                        ecb[:, :, c * 128 : (c + 1) * 128],
                        va_d[:, :, 0:1],
                        start=(c == 0),
                        stop=(c == NCH - 1),
                        perf_mode=PM.DoubleRow,
                        skip_group_check=True,
                    )
                # p = exp(scores) [128 t, 16 chunks] in one ACT instruction
                p_sb = p_pool.tile([128, NCH], dt.bfloat16, tag="p")
                nc.scalar.activation(p_sb[:], scb[:, 0:NCH], AF.Exp)
                # Z on every partition via ones^T @ p, then 1/Z, then p/Z
                zc = 32 + 16 * b
                nc.tensor.matmul(
                    ctp[0:128, zc : zc + NCH], ones_bf[:], p_sb[:],
                    start=(b == 0), stop=True, skip_group_check=True,
                )
                zw = dve_sm.tile([128, 1], dt.float32, tag="zw")
                nc.vector.tensor_reduce(
                    zw[:], ctp[0:128, zc : zc + NCH], axis=AX.X, op=OP.add
                )
                rz = dve_sm.tile([128, 1], dt.float32, tag="rz")
                nc.vector.reciprocal(rz[:], zw[:])
                pn = pn_pool.tile([128, NCH], dt.float8e4, tag="pn")
                nc.vector.tensor_scalar(
                    pn[:], p_sb[:], rz[:, 0:1], op0=OP.mult,
                    scalar2=256.0, op1=OP.mult,
                )
                pn_tiles[b] = pn

            def emit_context(b):
                """flipped context for batch b: encN chunk stationary, p_n
                column moving -> N=1 matmuls, ~zero PE engine cost."""
                pn = pn_tiles.pop(b)
                for m, mlo, msz in ((0, 0, H0), (1, H0, H1)):
                    for c in range(NCH):
                        nc.tensor.matmul(
                            ctp[0:msz, 16 * m + b : 16 * m + b + 1],
                            en_tiles[b][:, c, mlo : mlo + msz],
                            pn[:, c : c + 1],
                            start=False,
                            stop=(b == NB - 1 and m == 1 and c == NCH - 1),
                            skip_group_check=True,
                        )

            for b in range(NB):
                etd = etd_pool.tile([128, 2, T], dt.float8e4, tag="etd")
                if b == 0:
                    nc.sync.dma_start(etd[:], d_encTD[b])
                    nc.sync.dma_start(ua_d[:], d_UaTD[:, :, :])
                    nc.sync.dma_start(wa0[:], d_WaT[0:H0, :])
                    nc.sync.dma_start(qt0[:], d_qT[0:H0, :])
                    nc.sync.dma_start(wa1[:], d_WaT[H0:H, :])
                    nc.sync.dma_start(qt1[:], d_qT[H0:H, :])
                    nc.sync.dma_start(qb0[:], d_qb[0:H0, :])
                    nc.sync.dma_start(qb1[:], d_qb[H0:H, :])
                    nc.sync.dma_start(va_d[:], d_VaD[:, :, :])
                else:
                    nc.sync.dma_start(etd[:], d_encTD[b])  # noqa
                    # qprojT = Wa @ q^T + (ba + bua), in H0/H1 chunks
                    for m, mlo, msz, qb_m in ((0, 0, H0, qb0), (1, H0, H1, qb1)):
                        ps = sc_ps.tile([128, 512], dt.float32, tag="sc")
                        nc.tensor.matmul(
                            ps[0:msz, 0:NB], wa0[:, mlo : mlo + msz], qt0[:],
                            start=True, stop=False,
                        )
                        nc.tensor.matmul(
                            ps[0:msz, 0:NB], wa1[:, mlo : mlo + msz], qt1[:],
                            start=False, stop=True,
                        )
                        nc.vector.tensor_scalar_add(
                            qproj[m][:], ps[0:msz, 0:NB], qb_m[:]
                        )
                if b == 2:
                    # decoder weights after the first three encTD loads
                    nc.sync.dma_start(wihc0[:], d_WihcT[0:H0, :])
                    nc.sync.dma_start(wihc1[:], d_WihcT[H0 : H + 1, :])
                    nc.sync.dma_start(whh0[:], d_WhhT[0:H0, :])
                    nc.sync.dma_start(whh1[:], d_WhhT[H0:H, :])
                    nc.sync.dma_start(wxr_sb[:], d_wxr[:, :])
                    nc.sync.dma_start(w1t0[:], d_W1T[0:H0, :])
                    nc.sync.dma_start(w1t1[:], d_W1T[H0 : H + 1, :])
                    nc.sync.dma_start(w2t[:], d_W2T[:, :])
                    nc.sync.dma_start(w3t[:], d_W3T[:, :])
                    nc.sync.dma_start(c0_sb[:], d_c0[:, :])
                    nc.sync.dma_start(x_sb[:], d_x0[:, :])
                ec = ec_bufs[b % NEC]
                i_kp = None
                for m, mlo, msz in ((0, 0, H0), (1, H0, H1)):
                    for th in (0, 1):
                        kp = kp_ps.tile([128, 1024], dt.float32, tag="kp")
                        for n in (0, 1):
                            lo = n * 512
                            i_kp = nc.tensor.matmul(
                                kp[0:msz, lo : lo + 512],
                                ua_d[:, :, mlo : mlo + msz],
                                etd[:, :, th * 1024 + lo : th * 1024 + lo + 512],
                                start=True,
                                stop=True,
                                perf_mode=PM.DoubleRow,
                            )
                        nc.scalar.activation(
                            ec[0:msz, m, th * 1024 : (th + 1) * 1024],
                            kp[0:msz, :],
                            AF.Tanh,
                            bias=qproj[m][:, b : b + 1],
                        )
                # encN paced on the SWDGE ring behind this batch's kproj;
                # layout "(n p) h": partition = t % 128, chunk = t // 128 --
                # exactly the transposed-scores layout, so no pT transposes
                en = encN_pool.tile(
                    [128, NCH, H], dt.float8e4, name=f"en{b}", tag="en"
                )
                i_en = nc.gpsimd.dma_start(en[:], d_encN[b])
                _br.add_dep_helper(
                    i_en.ins, i_kp.ins, sync=True,
                    reason="encN paced behind this batch's kproj",
                )
                en_tiles.append(en)

                if b == 4:
                    # G0 q-part precomputed here (PE idle, weights landed);
                    # the ct-part accumulates on top after the last context
                    for n, nsz in [(0, 512), (512, G4 - 512)]:
                        nc.tensor.matmul(
                            gp[:, n : n + nsz], qt0[:], whh0[:, n : n + nsz],
                            start=True, stop=False, skip_group_check=True,
                        )
                        nc.tensor.matmul(
                            gp[:, n : n + nsz], qt1[:], whh1[:, n : n + nsz],
                            start=False, stop=False, skip_group_check=True,
                        )
                # scores/exp/Z for batch b-1: delayed one batch so PE
                # never parks on tanh(b) ahead of kproj(b+1)
                if b >= 1:
                    emit_scores(b - 1)
                # context for batch b-2 (encN(b-2) has landed by now)
                if b >= 2:
                    emit_context(b - 2)

            emit_scores(NB - 1)

            emit_context(NB - 2)
            emit_context(NB - 1)

            # assemble ct0/ct1 from the context PSUM bank (undo the 256x
            # fp8 range scaling of pn)
            nc.vector.tensor_scalar_mul(ct0[:], ctp[0:H0, 0:NB], 1.0 / 256.0)
            nc.vector.tensor_scalar_mul(ct1[0:H1, :], ctp[0:H1, 16 : 16 + NB], 1.0 / 256.0)

        # switch ACT tables to sigmoid_and_others (sigmoid+tanh+relu) while
        # PE/DVE finish G0; reading ct0 pins this after the last exp so the
        # scheduler cannot float it into the attention stream
        wt_c = spool.tile([1, 8], dt.float32)
        nc.scalar.activation(wt_c[:], ct0[0:1, 0:8], AF.Sigmoid)

        g0_bf = spool.tile([NB, G4], dt.bfloat16)
        for n, nsz in [(0, 512), (512, G4 - 512)]:
            nc.tensor.matmul(
                gp[:, n : n + nsz], ct0[:], wihc0[:, n : n + nsz],
                start=False, stop=False, skip_group_check=True,
            )
            nc.tensor.matmul(
                gp[:, n : n + nsz], ct1[:], wihc1[:, n : n + nsz],
                start=False, stop=True, skip_group_check=True,
            )
            if n == 0:
                nc.vector.tensor_copy(g0_bf[:, 0:512], gp[:, 0:512])
        # second half on ACT (Copy is in the sigmoid set), overlapping DVE
        nc.scalar.activation(g0_bf[:, 512:G4], gp[:, 512:G4], AF.Copy)

        # ---------- decoder: 5 serial steps ----------
        # gate order (host-permuted): i 0:200 | f 200:400 | o 400:600 | g 600:800
        ht0 = spool.tile([H0, NB], dt.bfloat16)
        ht1 = spool.tile([H1 + 1, NB], dt.bfloat16)  # row 72 = ones (b1 row)
        nc.sync.dma_start(ht1[H1 : H1 + 1, :], d_ones_b[:, :])
        o1t = spool.tile([101, NB], dt.bfloat16)  # row 100 = ones (b2 row)
        nc.sync.dma_start(o1t[100:101, :], d_ones_b[:, :])
        o2t = spool.tile([51, NB], dt.bfloat16)  # row 50 = ones (b3 row)
        nc.sync.dma_start(o2t[50:51, :], d_ones_b[:, :])

        with (
            tc.tile_pool(name="ls", bufs=2) as ls,
            tc.tile_pool(name="ls_ps0", bufs=1, space="PSUM") as lp0,
            tc.tile_pool(name="ls_ps1", bufs=1, space="PSUM") as lp1,
            tc.tile_pool(name="ls_ps2", bufs=1, space="PSUM") as lp2,
            tc.tile_pool(name="ls_ps3", bufs=1, space="PSUM") as lp3,
            tc.tile_pool(name="ls_ps4", bufs=1, space="PSUM") as lp4,
        ):
            xt = x_sb
            for t in range(NSTEPS):
                # gates = wxr*x + G0, fused per half so sigmoid(i,f) starts
                # as soon as the low half lands
                gates2 = ls.tile([NB, G4], dt.bfloat16, tag="gates2")
                nc.vector.scalar_tensor_tensor(
                    gates2[:, 0:400], wxr_sb[:, 0:400], xt[:, 0:1],
                    g0_bf[:, 0:400], op0=OP.mult, op1=OP.add,
                )
                nc.vector.scalar_tensor_tensor(
                    gates2[:, 400:G4], wxr_sb[:, 400:G4], xt[:, 0:1],
                    g0_bf[:, 400:G4], op0=OP.mult, op1=OP.add,
                )
                sfo = ls.tile([NB, 600], dt.float32, tag="sfo")
                nc.scalar.activation(sfo[:, 0:400], gates2[:, 0:400], AF.Sigmoid)
                g2 = ls.tile([NB, H], dt.float32, tag="g2")
                nc.scalar.activation(g2[:], gates2[:, 3 * H : 4 * H], AF.Tanh)
                nc.scalar.activation(sfo[:, 400:600], gates2[:, 2 * H : 3 * H], AF.Sigmoid)
                t1 = ls.tile([NB, H], dt.float32, tag="t1")
                nc.vector.tensor_tensor(t1[:], sfo[:, H : 2 * H], c0_sb[:], op=OP.mult)
                t2 = ls.tile([NB, H], dt.float32, tag="t2")
                nc.vector.tensor_tensor(t2[:], sfo[:, 0:H], g2[:], op=OP.mult)
                cn = ls.tile([NB, H], dt.float32, tag="cn")
                nc.vector.tensor_tensor(cn[:], t1[:], t2[:], op=OP.add)
                tcn = ls.tile([NB, H], dt.float32, tag="tcn")
                nc.scalar.activation(tcn[:], cn[:], AF.Tanh)
                hh = ls.tile([NB, H], dt.bfloat16, tag="hh")
                nc.vector.tensor_tensor(hh[:], sfo[:, 400:600], tcn[:], op=OP.mult)
                # transpose h -> feature-major; relu on ACT and DVE in parallel
                tp0 = lp0.tile([128, 1024], dt.bfloat16, tag="d0")
                nc.tensor.transpose(tp0[:, 0:NB], hh[:, 0:H0], id_bf[0:NB, 0:NB])
                nc.scalar.activation(ht0[:], tp0[:, 0:NB], AF.Relu)
                tp1 = lp1.tile([128, 1024], dt.bfloat16, tag="d1")
                nc.tensor.transpose(tp1[0:H1, 0:NB], hh[:, H0:H], id_bf[0:NB, 0:NB])
                nc.vector.tensor_scalar_max(ht1[0:H1, :], tp1[0:H1, 0:NB], 0.0)
                # MLP in feature-major, biases via ones rows; relus on DVE
                # (shorter ack than ACT, and ACT is serial with sig/tanh)
                m1 = lp2.tile([100, 512], dt.float32, tag="d2")
                nc.tensor.matmul(m1[:, 0:NB], w1t0[:], ht0[:], start=True, stop=False)
                nc.tensor.matmul(m1[:, 0:NB], w1t1[:], ht1[:], start=False, stop=True)
                nc.vector.tensor_scalar_max(o1t[0:100, :], m1[:, 0:NB], 0.0)
                m2 = lp3.tile([50, 512], dt.float32, tag="d3")
                nc.tensor.matmul(m2[:, 0:NB], w2t[:], o1t[:], start=True, stop=True)
                nc.vector.tensor_scalar_max(o2t[0:50, :], m2[:, 0:NB], 0.0)
                # y = o2t^T @ w3 -> [NB, 1]: feeds back as next x, no transpose
                y_ps = lp4.tile([NB, 512], dt.float32, tag="d4")
                nc.tensor.matmul(y_ps[:, 0:1], o2t[:], w3t[:], start=True, stop=True)
                nc.vector.tensor_copy(y_sb[:, t : t + 1], y_ps[:, 0:1])
                xt = y_sb[:, t : t + 1]

            nc.sync.dma_start(d_y[:, :], y_sb[:])

    # Bacc lowering: register allocation + wait splitting (<=1 wait/inst on HW)
    nc.compile()
    return nc


def _prep_inputs(x, h0, c0, encoder_output, Wa, ba, Ua, bua, Va, bva,
                 W_ih, W_hh, b_ih, b_hh, W1, b1, W2, b2, W3, b3):
    """Host-side layout prep -> list of per-core input maps."""
    f32 = np.float32
    enc = np.ascontiguousarray(encoder_output, dtype=f32)
    q = np.asarray(h0, dtype=f32)[0]          # [B, H]
    c0f = np.asarray(c0, dtype=f32)[0]        # [B, H]
    x0 = np.asarray(x, dtype=f32).reshape(B, 1)

    # gate permutation: torch order [i, f, g, o] -> [i, f, o, g]
    perm = np.concatenate([np.arange(0, 2 * H), np.arange(3 * H, 4 * H),
                           np.arange(2 * H, 3 * H)])

    W_ih_f = np.asarray(W_ih, f32)
    wihct = np.concatenate(
        [W_ih_f[:, 1:].T,
         (np.asarray(b_ih, f32) + np.asarray(b_hh, f32)).reshape(1, G4)],
        axis=0,
    )[:, perm]
    whht = np.ascontiguousarray(np.asarray(W_hh, f32).T)[:, perm]
    wxr = np.broadcast_to(W_ih_f[:, 0][perm].reshape(1, G4), (NB, G4))

    # DoubleRow pairings: contraction padded 200 -> 256, row h' = p + 128*i.
    # The m dim of ua_d is padded 200 -> 208 and va_d's pair stride to 16 so
    # the dual-fp8 LDWEIGHTS pair step is 16-byte aligned (ISA restriction).
    UaT = np.ascontiguousarray(np.asarray(Ua, f32).T)          # [h', h]
    UaTp = np.zeros((256, 208), f32); UaTp[:H, :H] = UaT
    ua_d = UaTp.reshape(2, 128, 208).transpose(1, 0, 2)        # [128, 2, 208]
    vap = np.zeros((256,), f32); vap[:H] = np.asarray(Va, f32)[0]
    va_d = np.zeros((128, 2, 16), f32)
    va_d[:, :, 0] = vap.reshape(2, 128).T

    shared = {
        "UaTD": np.ascontiguousarray(ua_d).astype(F8),
        "WaT": np.ascontiguousarray(np.asarray(Wa, f32).T).astype(BF16),
        "qb": (np.asarray(ba, f32) + np.asarray(bua, f32)).reshape(H, 1),
        "VaD": np.ascontiguousarray(va_d).astype(F8),
        "WihcT": wihct.astype(BF16),
        "WhhT": whht.astype(BF16),
        "wxr": np.ascontiguousarray(wxr).astype(BF16),
        "W1T": np.concatenate(
            [np.asarray(W1, f32).T, np.asarray(b1, f32).reshape(1, 100)], axis=0
        ).astype(BF16),
        "W2T": np.concatenate(
            [np.asarray(W2, f32).T, np.asarray(b2, f32).reshape(1, 50)], axis=0
        ).astype(BF16),
        "W3T": np.concatenate(
            [np.asarray(W3, f32).T, np.asarray(b3, f32).reshape(1, 1)], axis=0
        ).astype(BF16),
        "ones_b": np.ones((1, NB), BF16),
    }

    in_maps = []
    for c in range(NCORES):
        bs = slice(c * NB, (c + 1) * NB)
        enc_c = enc[bs]  # [NB, T, H]
        encT = enc_c.transpose(0, 2, 1)  # [NB, H, T]
        m = dict(shared)
        encTp = np.zeros((NB, 256, T), f32)
        encTp[:, :H] = encT
        m["encTD"] = np.ascontiguousarray(
            encTp.reshape(NB, 2, 128, T).transpose(0, 2, 1, 3)
        ).astype(F8)
        m["encN"] = np.ascontiguousarray(
            enc_c.reshape(NB, NCH, 128, H).transpose(0, 2, 1, 3)
        ).astype(F8)
        m["qT"] = np.ascontiguousarray(q[bs].T).astype(BF16)
        m["c0s"] = np.ascontiguousarray(c0f[bs])
        m["x0s"] = np.ascontiguousarray(x0[bs])
        in_maps.append(m)
    return in_maps


def kernel(**inputs):
    from concourse.bass_utils import run_bass_kernel_spmd

    if "nc" not in _CACHE:
        _CACHE["nc"] = _build_module()
    nc = _CACHE["nc"]

    in_maps = _prep_inputs(**inputs)
    res = run_bass_kernel_spmd(nc, in_maps, core_ids=list(range(NCORES)))
    # y per core: [NB, NSTEPS] -> full output [B, NSTEPS]
    out = np.concatenate([r["y"] for r in res.results], axis=0)
    return np.ascontiguousarray(out.astype(np.float32))
